# revision 19
# baseline (speedup 1.0000x reference)
"""Causal self-attention Trainium2 kernel (B=4, T=2048, E=1024, H=16, D=64).

Sharding: 8 cores = batch(4) x head-group(2). Each core computes the full
attention for 8 heads of one batch element plus its half of the output
projection; the host sums the two out-proj partials per batch element.

v2 dataflow (per core, all matmul operands bf16, PSUM f32):
  - Host pre-transposes/bf16-casts x and the weights so contraction dims
    land on partitions: xT [E,T], wqkvT [E,1536], woT [512,E]. All weights
    resident in SBUF for the whole kernel.
  - Projection runs per T-quarter and is interleaved with attention of the
    previous query tile so the PE never idles (keeps the HAM clock-gate at
    8/8) and ScalarE exp overlaps projection matmuls.
  - S^T chunks [128kv, 512q]: the two heads of a pair are issued as
    row-group-tiled matmul pairs (tile_position (0,0)/(64,0)) that run
    concurrently in the PE array. Diagonal chunks only compute live
    columns.
  - exp on ScalarE (scale=1/sqrt(D)) -> bf16; causal mask applied via a
    DVE multiply with a precomputed [128,640] zeros|tril constant.
  - y^T accumulation [65, 512] per head with lhsT = V_aug (ones column
    carries the softmax denominator through the PV matmul).
  - Denominator rows are staged to a [32,512] SBUF tile (DVE copy to
    partition 64 + partition-shifting SBUF DMA), reciprocal'd in one DVE
    reciprocal_approx_fast per query tile, gpsimd-broadcast, and applied
    as one in-place [128,512] DVE multiply per (pair, tile). ScalarE
    runs Exp only - no activation-table reloads anywhere.
  - Out-projection for tile j is emitted after attention tile j+1 so its
    PSUM matmuls never stall the PE stream; partials DMA out as bf16 and
    the host sums the two head-group halves in f32.
"""

import numpy as np
import ml_dtypes

import concourse.bass as bass
import concourse.bacc as bacc
import concourse.mybir as mybir
import concourse.tile as tile
from concourse import bass_utils

f32 = mybir.dt.float32
bf16 = mybir.dt.bfloat16
FP = mybir.dt.float32  # psum dtype

P = 128
B, T, E = 4, 2048, 1024
H, D = 16, 64
HPC = H // 2            # heads per core = 8
NE = E // P             # 8 e-chunks
NTT = T // P            # 16 kv chunks
NQ = T // 512           # 4 query tiles of 512
SCALE = 1.0 / np.sqrt(D)

Exp = mybir.ActivationFunctionType.Exp
MULT = mybir.AluOpType.mult
IS_GE = mybir.AluOpType.is_ge

_CACHE = {}


def build(**opts):
    nc = bacc.Bacc("TRN2", target_bir_lowering=False, debug=False, num_devices=8)

    xT_d = nc.dram_tensor("xT", [E, T], bf16, kind="ExternalInput")
    wqkvT_d = nc.dram_tensor("wqkvT", [E, 3 * 512], bf16, kind="ExternalInput")
    woT_d = nc.dram_tensor("woT", [512, E], bf16, kind="ExternalInput")
    mask_d = nc.dram_tensor("mask", [P, 640], bf16, kind="ExternalInput")
    out_d = nc.dram_tensor("out", [T, E], bf16, kind="ExternalOutput")

    with tile.TileContext(nc) as tc:
        build_body(tc, xT_d, wqkvT_d, woT_d, mask_d, out_d, **opts)
    nc.compile()
    return nc


def build_body(tc, xT_d, wqkvT_d, woT_d, mask_d, out_d,
               ptp_bufs=4, pss_bufs=2, psy_bufs=2):
    nc = tc.nc

    from contextlib import ExitStack
    with ExitStack() as top:
        per = top.enter_context(tc.tile_pool(name="per", bufs=1))

        qk_sb = per.tile([P, 8, T], bf16)            # chunks 0-3: Q^T, 4-7: K^T
        v_sb = per.tile([P, NTT, HPC, D + 1], bf16)  # [kv_p, kv_chunk, head, d|1]
        yt_sb = per.tile([P, 4, T], bf16)            # [f%128, f//128, q]
        wv_sb = per.tile([P, NE, 512], bf16)         # V-proj weights
        wqk_sb = per.tile([P, 8, NE, P], bf16)       # QK-proj weights per f-chunk
        wo_sb = per.tile([P, 4, E], bf16)            # out-proj weights
        mask_sb = per.tile([P, 640], bf16)           # zeros(512) | tril(128)
        # softmax denominators + reciprocals, one [8,512] tile per query
        # tile (ISA ops require APs that start at partition 0)
        l_js = [per.tile([8, 512], f32, name=f"l_sb{j}") for j in range(NQ)]
        rc_js = [per.tile([8, 512], f32, name=f"rc_sb{j}") for j in range(NQ)]
        # self-managed ptt ring: persistent tiles, zeroed once so the causal
        # mask-multiply never sees uninitialized data (NaN*0=NaN) in the
        # stale columns the diagonal chunks skip
        ptt_ring = [per.tile([P, 1024], bf16, name=f"ptt{k}")
                    for k in range(4)]
        for pt in ptt_ring:
            nc.vector.memset(pt, 0.0)
        ptt_ctr = [0]

        # --------- prologue DMAs (wv + first x quarter first, per-chunk
        # interleaved so the first V matmul can start ~1us in) ----------
        xpool = top.enter_context(tc.tile_pool(name="xpool", bufs=2))
        xts = {}

        def load_x_quarter(th):
            for e in range(NE):
                xt = xpool.tile([P, 512], bf16, tag=f"xt{e}")
                nc.sync.dma_start(
                    xt, xT_d[e * P:(e + 1) * P, th * 512:(th + 1) * 512])
                xts[(e, th)] = xt

        for e in range(NE):
            nc.sync.dma_start(
                wv_sb[:, e, :], wqkvT_d[e * P:(e + 1) * P, 1024:1536])
            xt = xpool.tile([P, 512], bf16, tag=f"xt{e}")
            nc.sync.dma_start(xt, xT_d[e * P:(e + 1) * P, 0:512])
            xts[(e, 0)] = xt
        for ft in range(8):
            nc.sync.dma_start(
                wqk_sb[:, ft],
                wqkvT_d[:, ft * P:(ft + 1) * P].rearrange("(o p) f -> p o f", p=P))
        nc.sync.dma_start(wo_sb, woT_d.rearrange("(o p) f -> p o f", p=P))
        nc.sync.dma_start(mask_sb, mask_d[:, :])

        psp = top.enter_context(tc.tile_pool(name="psp", bufs=2, space="PSUM"))
        drn = top.enter_context(tc.tile_pool(name="drn", bufs=2))
        nrm = top.enter_context(tc.tile_pool(name="nrm", bufs=2))
        ost = top.enter_context(tc.tile_pool(name="ost", bufs=2))
        pss = top.enter_context(
            tc.tile_pool(name="pss", bufs=pss_bufs, space="PSUM"))
        psy = top.enter_context(
            tc.tile_pool(name="psy", bufs=psy_bufs, space="PSUM"))

        def proj_v(th):
            # V projection for this quarter: natural layout [T, 512]
            for tti in range(4):
                tt = th * 4 + tti
                ps = psp.tile([P, 512], FP, tag="pp")
                for e in range(NE):
                    nc.tensor.matmul(
                        ps,
                        lhsT=xts[(e, th)][:, tti * P:(tti + 1) * P],
                        rhs=wv_sb[:, e, :],
                        start=(e == 0), stop=(e == NE - 1))
                nc.vector.tensor_copy(
                    v_sb[:, tt, :, 0:D],
                    ps.rearrange("p (h d) -> p h d", h=HPC))
            # ones column for this quarter (never keeps in_: cond<0)
            ov = v_sb[:, th * 4:(th + 1) * 4, :, D:D + 1]
            iv = v_sb[:, th * 4:(th + 1) * 4, :, 0:1]
            nc.gpsimd.affine_select(
                ov, iv, pattern=[[0, 4], [0, HPC], [0, 1]],
                compare_op=IS_GE, fill=1.0, base=-1,
                channel_multiplier=0)

        def proj_qk(th, fts):
            # QK^T projection for this quarter: [f, 512] layout
            for ft in fts:
                ps = psp.tile([P, 512], FP, tag="pp")
                for e in range(NE):
                    nc.tensor.matmul(
                        ps,
                        lhsT=wqk_sb[:, ft, e, :],
                        rhs=xts[(e, th)][:, :],
                        start=(e == 0), stop=(e == NE - 1))
                nc.vector.tensor_copy(
                    qk_sb[:, ft, th * 512:(th + 1) * 512], ps)

        def att_block(c, j):
            """Attention for head pair (2c, 2c+1), query tile j."""
            jsl = slice(j * 512, (j + 1) * 512)
            nkv = 4 * j + 4
            yps = [psy.tile([D + 1, 512], FP, tag="y", name=f"yps_{c}_{j}_{k}")
                   for k in range(2)]
            for i in range(nkv):
                off = i - 4 * j
                q0 = max(0, 128 * off)
                spt = pss.tile([P, 1024], FP, tag="s")
                for hh in range(2):
                    p0 = 64 * hh
                    nc.tensor.matmul(
                        spt[:, hh * 512 + q0:(hh + 1) * 512],
                        lhsT=qk_sb[p0:p0 + 64, 4 + c, i * P:(i + 1) * P],
                        rhs=qk_sb[p0:p0 + 64, c, j * 512 + q0:(j + 1) * 512],
                        start=True, stop=True,
                        tile_position=(p0, 0))
                ptt = ptt_ring[ptt_ctr[0] % 4]
                ptt_ctr[0] += 1
                if off < 0:
                    nc.scalar.activation(ptt, spt, Exp, scale=float(SCALE))
                else:
                    pv = ptt.rearrange("p (h q) -> p h q", h=2)
                    sv = spt.rearrange("p (h q) -> p h q", h=2)
                    nc.scalar.activation(pv[:, :, q0:512], sv[:, :, q0:512],
                                         Exp, scale=float(SCALE))
                    # causal mask: zero stale cols [0,q0) + triangle
                    # [q0,q0+128) via mask constant (zeros(512)|tril(128))
                    for hx in range(2):
                        nc.vector.tensor_tensor(
                            pv[:, hx, 0:q0 + P], pv[:, hx, 0:q0 + P],
                            mask_sb[:, 512 - q0:640], MULT)
                for hh in range(2):
                    nc.tensor.matmul(
                        yps[hh],
                        lhsT=v_sb[:, i, 2 * c + hh, :],
                        rhs=ptt[:, hh * 512:(hh + 1) * 512],
                        start=(i == 0), stop=(i == nkv - 1))
            # drain: unnormalized y -> yt_sb lower half / tmpb (upper half
            # staged until norm), denominators -> l_sb rows (l rows ride
            # partition 64 of sc, then partition-shifting SBUF DMAs place
            # them on l_sb rows for the batched recip)
            sc = drn.tile([D + 1, 1024], f32, tag="sc")
            nc.vector.tensor_copy(yt_sb[0:D, c, jsl], yps[0][0:D, :])
            nc.vector.tensor_copy(sc[D:D + 1, 0:512], yps[0][D:D + 1, :])
            tmpb = drn.tile([D, 512], bf16, tag="tmpb", bufs=5)
            nc.vector.tensor_copy(tmpb, yps[1][0:D, :])
            nc.vector.tensor_copy(sc[D:D + 1, 512:1024], yps[1][D:D + 1, :])
            nc.sync.dma_start(l_js[j][2 * c:2 * c + 1, :],
                              sc[D:D + 1, 0:512])
            nc.sync.dma_start(l_js[j][2 * c + 1:2 * c + 2, :],
                              sc[D:D + 1, 512:1024])
            tmpbs[(c, j)] = tmpb

        def norm(j):
            """Reciprocal + broadcast + normalize of yt tile j.

            partition_broadcast needs its source on partition 0, so the 8
            reciprocal rows of tile j are first gathered into a single
            partition-0 tile with one SBUF DMA."""
            jsl = slice(j * 512, (j + 1) * 512)
            nc.vector.reciprocal_approx_fast(rc_js[j][:, :], l_js[j][:, :])
            rcrow = nrm.tile([1, 8, 512], f32, tag="rcrow")
            nc.sync.dma_start(rcrow, rc_js[j][:, :])
            for c in range(4):
                bc0 = nrm.tile([D, 512], f32, tag="bc", bufs=3)
                nc.gpsimd.partition_broadcast(bc0, rcrow[0:1, 2 * c, :])
                nc.vector.tensor_tensor(
                    yt_sb[0:D, c, jsl], yt_sb[0:D, c, jsl], bc0, MULT)
                bc1 = nrm.tile([D, 512], f32, tag="bc", bufs=3)
                nc.gpsimd.partition_broadcast(bc1, rcrow[0:1, 2 * c + 1, :])
                tmpn = nrm.tile([D, 512], bf16, tag="tmpn")
                nc.vector.tensor_tensor(tmpn, tmpbs[(c, j)], bc1, MULT)
                nc.sync.dma_start(yt_sb[64:128, c, jsl], tmpn)

        def out_proj(j):
            for tti in range(4):
                tt = 4 * j + tti
                po = pss.tile([P, 1024], FP, tag="s")
                for half in range(2):
                    for c2 in range(4):
                        nc.tensor.matmul(
                            po[:, half * 512:(half + 1) * 512],
                            lhsT=yt_sb[:, c2, tt * P:(tt + 1) * P],
                            rhs=wo_sb[:, c2, half * 512:(half + 1) * 512],
                            start=(c2 == 0), stop=(c2 == 3))
                st = ost.tile([P, E], bf16, tag="st")
                nc.vector.tensor_copy(st, po)
                nc.sync.dma_start(out_d[tt * P:(tt + 1) * P, :], st)

        # ------------------- emission schedule -------------------
        # Attention tile j is ScalarE(exp)-paced; interleave independent PE
        # work (projection of quarter j+1, out-projections) between its
        # c-blocks so the PE never idles long enough to drop HAM to 4/8.
        tmpbs = {}
        proj_v(0)
        proj_qk(0, (0, 4, 1, 5, 2, 6, 3, 7))
        load_x_quarter(1)

        att_block(0, 0)
        proj_v(1)
        att_block(1, 0)
        proj_qk(1, (0, 4, 1, 5))
        att_block(2, 0)
        proj_qk(1, (2, 6, 3, 7))
        att_block(3, 0)
        load_x_quarter(2)
        norm(0)

        att_block(0, 1)
        proj_v(2)
        att_block(1, 1)
        proj_qk(2, (0, 4, 1, 5))
        att_block(2, 1)
        proj_qk(2, (2, 6, 3, 7))
        att_block(3, 1)
        load_x_quarter(3)
        norm(1)

        att_block(0, 2)
        proj_v(3)
        att_block(1, 2)
        proj_qk(3, (0, 4, 1, 5))
        att_block(2, 2)
        proj_qk(3, (2, 6, 3, 7))
        att_block(3, 2)
        norm(2)

        att_block(0, 3)
        out_proj(0)
        att_block(1, 3)
        out_proj(1)
        att_block(2, 3)
        out_proj(2)
        att_block(3, 3)
        norm(3)
        out_proj(3)


def _shard_inputs(x, w_qkv, w_out):
    mask = np.zeros((P, 640), dtype=np.float32)
    mask[:, 512:640] = np.tril(np.ones((P, P), dtype=np.float32)).T
    mask = mask.astype(ml_dtypes.bfloat16)
    in_maps = []
    for core in range(8):
        b, hg = core // 2, core % 2
        sl = slice(hg * 512, (hg + 1) * 512)
        wq = w_qkv[0:1024][sl]
        wk = w_qkv[1024:2048][sl]
        wv = w_qkv[2048:3072][sl]
        wqkvT = np.ascontiguousarray(
            np.concatenate([wq, wk, wv], axis=0).T).astype(ml_dtypes.bfloat16)
        in_maps.append({
            "xT": np.ascontiguousarray(x[b].T).astype(ml_dtypes.bfloat16),
            "wqkvT": wqkvT,
            "woT": np.ascontiguousarray(
                w_out[:, sl].T).astype(ml_dtypes.bfloat16),
            "mask": mask,
        })
    return in_maps


def kernel(x, w_qkv, w_out, _trace=False):
    x = np.asarray(x, dtype=np.float32)
    w_qkv = np.asarray(w_qkv, dtype=np.float32)
    w_out = np.asarray(w_out, dtype=np.float32)

    if "nc" not in _CACHE:
        _CACHE["nc"] = build()
    nc = _CACHE["nc"]

    in_maps = _shard_inputs(x, w_qkv, w_out)
    res = bass_utils.run_bass_kernel_spmd(
        nc, in_maps, core_ids=list(range(8)), trace=_trace)
    kernel.last_result = res

    out = np.empty((B, T, E), dtype=np.float32)
    for b in range(B):
        out[b] = (res.results[2 * b]["out"].astype(np.float32)
                  + res.results[2 * b + 1]["out"].astype(np.float32))
    return out
